# revision 21
# baseline (speedup 1.0000x reference)
"""Trainium2 Bass kernel for nn_DecoderModel (12-layer decoder w/ image token).

Sharding: DP2 x TP4.  Cores 0-3 own batch 0, cores 4-7 own batch 1 (512
tokens each).  Megatron TP within each 4-core group:
  - qkv column-sharded (4 heads/core), proj row-sharded + group AllReduce
  - fc column-sharded (1024 dff/core), fc2 row-sharded + group AllReduce
  - lm head: vocab/4 per core for the group's 512 tokens (host assembles)

2-chunk causal pipeline: the 512 tokens are split into two 256-token
chunks.  Chunk 0 attends only to keys 0-255 (+image), so its
attn->proj->AllReduce->ln2->fc->fc2->AllReduce chain is independent of
chunk 1 within a layer; interleaving the two chunks keeps the collective
engine saturated (4 x 0.5MB AllReduce per layer, ~18.8us each = the
pipeline period) while all matmul work hides underneath, and keeps the
PE at max p-state.

Residual kept feature-major (h^T: [D, tok]).  LayerNorm folded into the
matmuls: y = r .* (x @ W - mu * colsum(W)) with gamma folded into W
host-side; the -mu*colsum term is a K=1 matmul into the same PSUM.

Attention: kv order is [tokens 0..511, image] (order inside softmax is
irrelevant).  V is built token-major (tokens on partitions) by swapping
stationary/moving in the matmul.  Scores are kt-major; causal structure
= per-key-tile column slicing plus one shared [128,128] triangle mask on
the diagonal block.  Denominators come from an appended attention-mask
column in V; the 4 per-head denominator broadcasts are fused into 2
block-select matmuls per chunk.
"""

import os
import numpy as np

from concourse import bacc, tile, mybir
from concourse import bass_utils

dt = mybir.dt
AF = mybir.ActivationFunctionType
ALU = mybir.AluOpType

# Model dims (hardcoded per contract)
B, S, D, H, L, V = 2, 512, 1024, 16, 12, 50257
HD = D // H          # 64
DFF = 4 * D          # 4096
NC = 8               # cores
TP = 4               # tensor-parallel group size
TOK = S              # tokens per core (= its batch's 512)
NH = H // TP         # 4 local heads
QC = NH * HD         # 256 q/k/v cols per core
DFS = DFF // TP      # 1024 dff cols per core
PRJ = QC             # 256 proj rows per core
VSH = (V + TP - 1) // TP   # 12565 vocab rows per core
VS = 12800           # padded vocab shard (25*512)
NVT = VS // 512      # 25 vocab tiles
EPS = 1e-5
EXPB = -2.0          # exp(s + EXPB): cancels in normalization; f16 headroom
CW = TOK // 2        # chunk width (tokens per pipeline chunk)

F32 = dt.float32
F16 = dt.float16

GROUPS = [[0, 1, 2, 3], [4, 5, 6, 7]]


def _build(nl):
    nc = bacc.Bacc("TRN2", target_bir_lowering=False, debug=False,
                   num_devices=NC)

    dram = lambda n, sh, ty=F16, kind="ExternalInput": nc.dram_tensor(
        n, sh, ty, kind=kind).ap()

    h0T_d = dram("h0T", [D, TOK])
    wqk_d = dram("wqk", [nl, D, 512])
    csqk_d = dram("csqk", [nl, 1, 512])
    wv_d = dram("wv", [nl, D, QC])
    csv_d = dram("csv", [nl, 1, QC])
    kiv_d = dram("kiv", [nl, QC, 1])
    viv_d = dram("viv", [nl, 1, NH * 65])
    wproj_d = dram("wproj", [nl, PRJ, D])
    wfc_d = dram("wfc", [nl, D, DFS])
    csfc_d = dram("csfc", [nl, 1, DFS])
    wfc2_d = dram("wfc2", [nl, DFS, D])
    tri_d = dram("tri", [128, 128])
    ambc_d = dram("ambc", [128, 4])
    onesq_d = dram("onesq", [128, 128])
    wteT_d = dram("wteT", [D, VS])
    logits_d = dram("logits", [TOK, VS], kind="ExternalOutput")

    CR = [slice(0, CW), slice(CW, TOK)]

    with tile.TileContext(nc) as tc:
        with (
            nc.allow_low_precision(reason="f16 pipeline"),
            tc.tile_pool(name="const", bufs=1) as cpool,
            tc.tile_pool(name="resid", bufs=1) as hpool,
            tc.tile_pool(name="rows", bufs=2) as rpool,
            tc.tile_pool(name="dram", bufs=1, space="DRAM") as dpool,
        ):
            ones_sb = cpool.tile([128, 128], F16, name="ones_sb")
            nc.sync.dma_start(ones_sb[:], onesq_d[:])
            ones_col = ones_sb[:, 0:1]
            ones_row = ones_sb[0:1, :]
            tri_sb = cpool.tile([128, 128], F16, name="tri_sb")
            nc.sync.dma_start(tri_sb[:], tri_d[:])
            ambsb = cpool.tile([128, 4], F16, name="ambsb")
            nc.sync.dma_start(ambsb[:], ambc_d[:])
            c_eps = cpool.tile([1, 1], F32, name="c_eps")
            nc.vector.memset(c_eps[:], EPS)
            c_invD = cpool.tile([1, 1], F32, name="c_invD")
            nc.vector.memset(c_invD[:], 1.0 / D)
            c_ninvD = cpool.tile([1, 1], F32, name="c_ninvD")
            nc.vector.memset(c_ninvD[:], -1.0 / D)
            c_negb = cpool.tile([128, 1], F32, name="c_negb")
            nc.vector.memset(c_negb[:], EXPB)

            # residual stream, 8 feature chunks [128, TOK]
            hT = []
            for kc in range(8):
                t_ = hpool.tile([128, TOK], F16, name=f"hT{kc}")
                nc.sync.dma_start(t_[:], h0T_d[kc * 128:(kc + 1) * 128, :])
                hT.append(t_)

            # token-major V with per-head [*,65] blocks (col 64 = attn mask)
            v5 = []
            for tc_ in range(4):
                v_ = hpool.tile([128, NH * 65], F16, name=f"v5_{tc_}")
                for h in range(NH):
                    nc.sync.dma_start(v_[:, h * 65 + 64:h * 65 + 65],
                                      ambc_d[:, tc_:tc_ + 1])
                v5.append(v_)

            with (
                tc.tile_pool(name="wts", bufs=2) as wpool,
                tc.tile_pool(name="act", bufs=1) as apool,
                tc.tile_pool(name="scratch", bufs=2) as spool,
                tc.tile_pool(name="ps_mm", bufs=3, space="PSUM") as ps_mm,
                tc.tile_pool(name="ps_s", bufs=3, space="PSUM") as ps_s,
                tc.tile_pool(name="ps_row", bufs=1, space="PSUM") as ps_row,
            ):
                # persistent activation tiles (written/read in chunk slices)
                q_sb = [apool.tile([128, TOK], F16, name=f"q{i}")
                        for i in range(2)]
                kT_sb = [apool.tile([128, S + 1], F16, name=f"kT{i}")
                         for i in range(2)]
                oT_sb = [apool.tile([128, TOK], F16, name=f"oT{i}")
                         for i in range(2)]
                g_sb = [apool.tile([128, TOK], F16, name=f"g{cc}")
                        for cc in range(8)]

                def resid_add(arout, c):
                    """hT[:, chunk c] += arout ([1024, CW] in DRAM)."""
                    cr = CR[c]
                    zs = []
                    for kc in range(8):
                        z = spool.tile([128, CW], F16, tag="zz", bufs=4)
                        nc.sync.dma_start(
                            z[:], arout[kc * 128:(kc + 1) * 128, :])
                        zs.append(z)
                    for kc in range(8):
                        eng = nc.gpsimd if kc % 2 == 0 else nc.vector
                        eng.tensor_tensor(hT[kc][:, cr], hT[kc][:, cr],
                                          zs[kc][:], ALU.add)

                def ln_stats(pfx, c, want_rT):
                    """LN stats over hT[:, chunk c].  Returns (nm [1,CW]
                    f16, rb_sb [128,CW] f32, rT_eff [128,2] f32|None)."""
                    cr = CR[c]
                    mu_ps = ps_row.tile([1, CW], F32, tag="rowA", bufs=1)
                    for kc in range(8):
                        nc.tensor.matmul(mu_ps[:], ones_col, hT[kc][:, cr],
                                         start=(kc == 0), stop=(kc == 7))
                    ssq_ps = ps_row.tile([1, CW], F32, tag="rowB", bufs=1)
                    for kc in range(8):
                        xsq = spool.tile([128, CW], F16, tag="xsq", bufs=3)
                        nc.scalar.activation(xsq[:], hT[kc][:, cr],
                                             AF.Square)
                        nc.tensor.matmul(ssq_ps[:], ones_col, xsq[:],
                                         start=(kc == 0), stop=(kc == 7))
                    musq = rpool.tile([1, CW], F32, tag="musq", bufs=2)
                    nc.scalar.activation(musq[:], mu_ps[:], AF.Square,
                                         scale=c_invD[:])
                    varr = rpool.tile([1, CW], F32, tag="varr", bufs=2)
                    nc.vector.scalar_tensor_tensor(
                        varr[:], ssq_ps[:], 1.0 / D, musq[:],
                        ALU.mult, ALU.subtract)
                    sd = rpool.tile([1, CW], F32, tag="sd", bufs=2)
                    nc.scalar.activation(sd[:], varr[:], AF.Sqrt,
                                         bias=c_eps[:])
                    rr = rpool.tile([1, CW], F32, tag="rr", bufs=2)
                    nc.vector.reciprocal_approx_fast(rr[:], sd[:])
                    r16 = rpool.tile([1, CW], F16, tag="r16", bufs=2)
                    nc.scalar.copy(r16[:], rr[:])
                    nm = rpool.tile([1, CW], F16, tag="nm", bufs=4)
                    nc.scalar.mul(nm[:], mu_ps[:], c_ninvD[:])
                    rb_ps = ps_mm.tile([128, CW], F32, tag="mm")
                    nc.tensor.matmul(rb_ps[:], ones_row, r16[:],
                                     start=True, stop=True)
                    rb_sb = spool.tile([128, CW], F32, tag=f"rb{pfx}",
                                       bufs=2)
                    nc.scalar.copy(rb_sb[:], rb_ps[:])
                    rT_eff = None
                    if want_rT:
                        rt_ps = ps_row.tile([128, 2], F32, tag="rowB",
                                            bufs=1)
                        for t in range(2):
                            nc.tensor.matmul(
                                rt_ps[:, t:t + 1],
                                r16[0:1, t * 128:(t + 1) * 128],
                                ones_row[0:1, 0:1],
                                start=True, stop=True,
                                skip_group_check=True)
                        rt_sb = rpool.tile([128, 2], F32, tag="rt", bufs=2)
                        nc.scalar.copy(rt_sb[:], rt_ps[:])
                        rT_eff = rpool.tile([128, 2], F32, tag="rte",
                                            bufs=2)
                        nc.vector.tensor_tensor(
                            rT_eff[:], rt_sb[:],
                            ambsb[:, 2 * c:2 * c + 2], ALU.mult)
                    return nm, rb_sb, rT_eff

                def qkv(l, c, nm1, rb1, rT1, wqk_sb, csqk_sb, wv_sb,
                        csv_sb):
                    cr = CR[c]
                    # q then k chains (each 128 cols of wqk)
                    for cc in range(4):
                        csl = slice(cc * 128, (cc + 1) * 128)
                        ps = ps_mm.tile([128, CW], F32, tag="mm")
                        for kc in range(8):
                            nc.tensor.matmul(ps[:], wqk_sb[kc][:, csl],
                                             hT[kc][:, cr],
                                             start=(kc == 0), stop=False)
                        nc.tensor.matmul(ps[:], csqk_sb[:, csl], nm1[:],
                                         start=False, stop=True)
                        if cc < 2:
                            out = q_sb[cc][:, cr]
                        else:
                            out = kT_sb[cc - 2][:, cr]
                        nc.vector.tensor_tensor(out, ps[:], rb1[:],
                                                ALU.mult)
                    # v chains, token-major (stationary = h token chunk)
                    for t in range(2):
                        tc_ = 2 * c + t
                        tsl = slice(tc_ * 128, (tc_ + 1) * 128)
                        ps = ps_mm.tile([128, CW], F32, tag="mm")
                        for kc in range(8):
                            nc.tensor.matmul(ps[:, 0:QC],
                                             hT[kc][:, tsl], wv_sb[kc][:],
                                             start=(kc == 0), stop=False)
                        nc.tensor.matmul(ps[:, 0:QC],
                                         nm1[0:1, t * 128:(t + 1) * 128],
                                         csv_sb[:], start=False, stop=True)
                        nc.vector.tensor_scalar(
                            v5[tc_].rearrange("p (h w) -> p h w",
                                              h=NH)[:, :, 0:64],
                            ps[:, 0:QC].rearrange("p (h w) -> p h w",
                                                  h=NH),
                            rT1[:, t:t + 1], None, ALU.mult)

                def attn(l, c, viv_sb):
                    """Attention for query chunk c (key tiles 0..2c+1 +
                    image), writing normalized oT slices."""
                    cr = CR[c]
                    nkt = 2 * c + 2      # key tiles visible to this chunk
                    den_ps = {}
                    o_raw = {}
                    p_tiles = {}

                    def scores_head(h):
                        qt = q_sb[h // 2]
                        kt = kT_sb[h // 2]
                        hsl = slice((h % 2) * 64, (h % 2) * 64 + 64)
                        pl = []
                        for ktile in range(nkt):
                            co = max(0, (ktile - 2 * c) * 128)
                            sps = ps_s.tile([128, CW], F32, tag="s")
                            nc.tensor.matmul(
                                sps[:, co:CW],
                                kt[hsl, ktile * 128:(ktile + 1) * 128],
                                qt[hsl, c * CW + co:(c + 1) * CW],
                                start=True, stop=True)
                            p = spool.tile([128, CW], F16, tag="p", bufs=8)
                            if ktile >= 2 * c:
                                # diagonal block: exp then triangle mask
                                ed = spool.tile([128, 128], F16, tag="ed",
                                                bufs=2)
                                nc.scalar.activation(
                                    ed[:], sps[:, co:co + 128],
                                    AF.Exp, bias=c_negb[:])
                                nc.vector.tensor_tensor(
                                    p[:, co:co + 128], ed[:],
                                    tri_sb[:], ALU.mult)
                                if co + 128 < CW:
                                    nc.scalar.activation(
                                        p[:, co + 128:CW],
                                        sps[:, co + 128:CW],
                                        AF.Exp, bias=c_negb[:])
                            else:
                                nc.scalar.activation(
                                    p[:], sps[:], AF.Exp, bias=c_negb[:])
                            pl.append((co, p))
                        simg = ps_row.tile([1, CW], F32,
                                           tag=("rowA", "rowB")[h % 2],
                                           bufs=1)
                        nc.tensor.matmul(simg[:], kt[hsl, S:S + 1],
                                         qt[hsl, cr], start=True,
                                         stop=True)
                        pimg = spool.tile([1, CW], F16, tag="pimg",
                                          bufs=2)
                        nc.scalar.activation(pimg[:], simg[:], AF.Exp,
                                             bias=c_negb[0:1, :])
                        p_tiles[h] = (pl, pimg)

                    def o_head(h):
                        pl, pimg = p_tiles[h]
                        ops = ps_mm.tile([128, CW], F32, tag="mm")
                        for ktile in range(nkt):
                            co, p = pl[ktile]
                            nc.tensor.matmul(
                                ops[0:65, co:CW],
                                v5[ktile][:, h * 65:(h + 1) * 65],
                                p[:, co:CW],
                                start=(ktile == 0), stop=False,
                                skip_group_check=True)
                        nc.tensor.matmul(
                            ops[0:65, :],
                            viv_sb[0:1, h * 65:(h + 1) * 65],
                            pimg[:], start=False, stop=True,
                            skip_group_check=True)
                        oraw = spool.tile([65, CW], F16, tag="oraw",
                                          bufs=4)
                        if h % 2 == 0:
                            nc.scalar.copy(oraw[:], ops[0:65, :])
                        else:
                            nc.vector.tensor_copy(oraw[:], ops[0:65, :])
                        o_raw[h] = oraw

                    scores_head(0)
                    scores_head(1)
                    o_head(0)
                    scores_head(2)
                    o_head(1)
                    scores_head(3)
                    o_head(2)
                    o_head(3)

                    # per-head denominator reciprocal broadcast [64, CW]
                    for h in range(NH):
                        den = rpool.tile([1, CW], F32, tag="den", bufs=2)
                        nc.scalar.copy(den[:], o_raw[h][64:65, :])
                        rcp = rpool.tile([1, CW], F32, tag="rcp", bufs=2)
                        nc.vector.reciprocal_approx_fast(rcp[:], den[:])
                        rch = rpool.tile([1, CW], F16, tag="rch", bufs=2)
                        nc.scalar.copy(rch[:], rcp[:])
                        rbps = ps_mm.tile([128, CW], F32, tag="mm")
                        nc.tensor.matmul(rbps[0:64, :],
                                         ones_row[0:1, 0:64], rch[:],
                                         start=True, stop=True)
                        rbc = spool.tile([64, CW], F32, tag="rbc",
                                         bufs=2)
                        nc.scalar.copy(rbc[:], rbps[0:64, :])
                        hsl = slice((h % 2) * 64, (h % 2) * 64 + 64)
                        nc.vector.tensor_tensor(
                            oT_sb[h // 2][hsl, cr], o_raw[h][0:64, :],
                            rbc[:], ALU.mult)

                def proj_ar(l, c, wproj_sb):
                    cr = CR[c]
                    arin = dpool.tile([D, CW], F16, name=f"aina{l}_{c}")
                    arout = dpool.tile([D, CW], F16, name=f"aouta{l}_{c}")
                    for mc in range(8):
                        msl = slice(mc * 128, (mc + 1) * 128)
                        zps = ps_mm.tile([128, CW], F32, tag="mm")
                        nc.tensor.matmul(zps[:], wproj_sb[0][:, msl],
                                         oT_sb[0][:, cr], start=True,
                                         stop=False)
                        nc.tensor.matmul(zps[:], wproj_sb[1][:, msl],
                                         oT_sb[1][:, cr], start=False,
                                         stop=True)
                        zsb = spool.tile([128, CW], F16, tag="ardrain",
                                         bufs=4)
                        if mc % 2 == 0:
                            nc.scalar.copy(zsb[:], zps[:])
                        else:
                            nc.vector.tensor_copy(zsb[:], zps[:])
                        nc.sync.dma_start(arin[msl, :], zsb[:])
                    nc.gpsimd.collective_compute(
                        "AllReduce", ALU.add, replica_groups=GROUPS,
                        ins=[arin.opt()], outs=[arout.opt()])
                    return arout

                def fc_fc2_ar(l, c, nm2, rb2, wfc_sb, csfc_sb, wfc2_sb):
                    cr = CR[c]
                    for cc in range(8):
                        csl = slice(cc * 128, (cc + 1) * 128)
                        ps = ps_mm.tile([128, CW], F32, tag="mm")
                        for kc in range(8):
                            nc.tensor.matmul(ps[:], wfc_sb[kc][:, csl],
                                             hT[kc][:, cr],
                                             start=(kc == 0), stop=False)
                        nc.tensor.matmul(ps[:], csfc_sb[:, csl], nm2[:],
                                         start=False, stop=True)
                        pre = spool.tile([128, CW], F32, tag="pre",
                                         bufs=2)
                        nc.vector.tensor_tensor(pre[:], ps[:], rb2[:],
                                                ALU.mult)
                        nc.scalar.activation(g_sb[cc][:, cr], pre[:],
                                             AF.Gelu_apprx_tanh)
                    arin = dpool.tile([D, CW], F16, name=f"ainm{l}_{c}")
                    arout = dpool.tile([D, CW], F16, name=f"aoutm{l}_{c}")
                    for mc in range(8):
                        msl = slice(mc * 128, (mc + 1) * 128)
                        zps = ps_mm.tile([128, CW], F32, tag="mm")
                        for kc in range(8):
                            nc.tensor.matmul(zps[:], wfc2_sb[kc][:, msl],
                                             g_sb[kc][:, cr],
                                             start=(kc == 0),
                                             stop=(kc == 7))
                        zsb = spool.tile([128, CW], F16, tag="ardrain",
                                         bufs=4)
                        if mc % 2 == 0:
                            nc.scalar.copy(zsb[:], zps[:])
                        else:
                            nc.vector.tensor_copy(zsb[:], zps[:])
                        nc.sync.dma_start(arin[msl, :], zsb[:])
                    nc.gpsimd.collective_compute(
                        "AllReduce", ALU.add, replica_groups=GROUPS,
                        ins=[arin.opt()], outs=[arout.opt()])
                    return arout

                arout_m_prev = [None, None]   # per chunk
                for l in range(nl):
                    # ---- weights for this layer
                    wqk_sb = []
                    for kc in range(8):
                        w = wpool.tile([128, 512], F16, tag=f"wqk{kc}",
                                       name=f"wqk{kc}_{l}")
                        nc.sync.dma_start(
                            w[:], wqk_d[l, kc * 128:(kc + 1) * 128, :])
                        wqk_sb.append(w)
                    csqk_sb = wpool.tile([1, 512], F16, tag="csqk",
                                         name=f"csqk_{l}")
                    nc.sync.dma_start(csqk_sb[:], csqk_d[l])
                    wv_sb = []
                    for kc in range(8):
                        w = wpool.tile([128, QC], F16, tag=f"wv{kc}",
                                       name=f"wv{kc}_{l}")
                        nc.sync.dma_start(
                            w[:], wv_d[l, kc * 128:(kc + 1) * 128, :])
                        wv_sb.append(w)
                    csv_sb = wpool.tile([1, QC], F16, tag="csv",
                                        name=f"csv_{l}")
                    nc.sync.dma_start(csv_sb[:], csv_d[l])
                    viv_sb = wpool.tile([1, NH * 65], F16, tag="viv",
                                        name=f"viv_{l}")
                    nc.sync.dma_start(viv_sb[:], viv_d[l])
                    wproj_sb = []
                    for kc in range(2):
                        w = wpool.tile([128, D], F16, tag=f"wproj{kc}",
                                       name=f"wproj{kc}_{l}")
                        nc.sync.dma_start(
                            w[:], wproj_d[l, kc * 128:(kc + 1) * 128, :])
                        wproj_sb.append(w)
                    wfc_sb = []
                    for kc in range(8):
                        w = wpool.tile([128, DFS], F16, tag=f"wfc{kc}",
                                       name=f"wfc{kc}_{l}")
                        nc.sync.dma_start(
                            w[:], wfc_d[l, kc * 128:(kc + 1) * 128, :])
                        wfc_sb.append(w)
                    csfc_sb = wpool.tile([1, DFS], F16, tag="csfc",
                                         name=f"csfc_{l}")
                    nc.sync.dma_start(csfc_sb[:], csfc_d[l])
                    wfc2_sb = []
                    for kc in range(8):
                        w = wpool.tile([128, D], F16, tag=f"wfc2{kc}",
                                       name=f"wfc2{kc}_{l}")
                        nc.sync.dma_start(
                            w[:], wfc2_d[l, kc * 128:(kc + 1) * 128, :])
                        wfc2_sb.append(w)

                    # image k columns for this layer
                    for i in range(2):
                        nc.sync.dma_start(
                            kT_sb[i][:, S:S + 1],
                            kiv_d[l, i * 128:(i + 1) * 128, :])

                    arout_a = [None, None]
                    # ---- A blocks: resid + ln1 + qkv + attn + proj + AR
                    for c in range(2):
                        if arout_m_prev[c] is not None:
                            resid_add(arout_m_prev[c], c)
                        nm1, rb1, rT1 = ln_stats("a", c, True)
                        qkv(l, c, nm1, rb1, rT1, wqk_sb, csqk_sb,
                            wv_sb, csv_sb)
                        attn(l, c, viv_sb)
                        arout_a[c] = proj_ar(l, c, wproj_sb)

                    # ---- B blocks: resid + ln2 + fc + fc2 + AR
                    arout_m = [None, None]
                    for c in range(2):
                        resid_add(arout_a[c], c)
                        nm2, rb2, _ = ln_stats("m", c, False)
                        arout_m[c] = fc_fc2_ar(l, c, nm2, rb2, wfc_sb,
                                               csfc_sb, wfc2_sb)
                    arout_m_prev = arout_m

                # ---- final LN (per chunk) -> xf
                xf = [[hpool.tile([128, CW], F16, name=f"xf{c}_{kc}")
                       for kc in range(8)] for c in range(2)]
                for c in range(2):
                    cr = CR[c]
                    resid_add(arout_m_prev[c], c)
                    nmf, rbf, _ = ln_stats("f", c, False)
                    nmr = rpool.tile([1, CW], F16, tag="nmr", bufs=2)
                    nc.vector.tensor_tensor(nmr[:], nmf[:],
                                            rbf[0:1, :], ALU.mult)
                    mrb_ps = ps_mm.tile([128, CW], F32, tag="mm")
                    nc.tensor.matmul(mrb_ps[:], ones_row, nmr[:],
                                     start=True, stop=True)
                    mrb = spool.tile([128, CW], F32, tag="mrb", bufs=2)
                    nc.scalar.copy(mrb[:], mrb_ps[:])
                    for kc in range(8):
                        nc.vector.tensor_tensor(xf[c][kc][:],
                                                hT[kc][:, cr], rbf[:],
                                                ALU.mult)
                        nc.vector.tensor_tensor(xf[c][kc][:],
                                                xf[c][kc][:], mrb[:],
                                                ALU.add)

            # ================= LM head =================
            with (
                tc.tile_pool(name="lm_w", bufs=3) as lwpool,
                tc.tile_pool(name="lm_sc", bufs=4) as lspool,
                tc.tile_pool(name="ps_lm", bufs=6, space="PSUM") as ps_lm,
            ):
                for vt in range(NVT):
                    vsl = slice(vt * 512, (vt + 1) * 512)
                    wt_sb = []
                    for kc in range(8):
                        w = lwpool.tile([128, 512], F16, tag=f"wte{kc}",
                                        name=f"wte{kc}_{vt}")
                        nc.sync.dma_start(
                            w[:], wteT_d[kc * 128:(kc + 1) * 128, vsl])
                        wt_sb.append(w)
                    for tcc in range(4):
                        csl = slice(tcc * 128, (tcc + 1) * 128)
                        lsl = slice((tcc % 2) * 128, (tcc % 2 + 1) * 128)
                        lg = ps_lm.tile([128, 512], F32, tag="lg")
                        for kc in range(8):
                            nc.tensor.matmul(lg[:],
                                             xf[tcc // 2][kc][:, lsl],
                                             wt_sb[kc][:],
                                             start=(kc == 0),
                                             stop=(kc == 7))
                        lsb = lspool.tile([128, 512], F16, tag="lmdrain",
                                          bufs=4)
                        if tcc % 2 == 0:
                            nc.scalar.copy(lsb[:], lg[:])
                        else:
                            nc.vector.tensor_copy(lsb[:], lg[:])
                        nc.sync.dma_start(logits_d[csl, vsl], lsb[:])

    nc.compile()
    return nc


def _prep(inputs):
    """Host-side preprocessing. Returns (in_maps, nl)."""
    f = lambda x: np.asarray(x, dtype=np.float32)
    ids = np.asarray(inputs["input_ids"]).astype(np.int64)
    am = f(inputs["attention_mask"])
    ihs = f(inputs["image_hidden_states"])
    wte = f(inputs["wte"])
    ft_W1, ft_b1 = f(inputs["ft_W1"]), f(inputs["ft_b1"])
    ft_W2, ft_b2 = f(inputs["ft_W2"]), f(inputs["ft_b2"])
    ln1_g = f(inputs["ln1_g"])
    Wattn = f(inputs["Wattn"])
    Wuk, buk = f(inputs["Wuk"]), f(inputs["buk"])
    Wuv, buv = f(inputs["Wuv"]), f(inputs["buv"])
    Wproj = f(inputs["Wproj"])
    ln2_g = f(inputs["ln2_g"])
    Wfc = f(inputs["Wfc"])
    Wfc2 = f(inputs["Wfc2"])
    lnf_g = f(inputs["lnf_g"])

    nl = int(os.environ.get("BASS_NLAYERS", str(L)))

    # embedding + image transform
    h0 = wte[ids.reshape(-1)] + np.tile(wte[:S], (B, 1))  # [B*S, D]
    h0T = np.ascontiguousarray(h0.T)
    img = np.maximum(ihs @ ft_W1 + ft_b1, 0.0) @ ft_W2 + ft_b2  # [B, D]

    # image k/v for all layers: [nl, B, D]
    ki = np.einsum("bd,ldm->lbm", img, Wuk[:nl]) + buk[:nl][:, None, :]
    vi = np.einsum("bd,ldm->lbm", img, Wuv[:nl]) + buv[:nl][:, None, :]

    tri = np.triu(np.ones((128, 128), np.float16))
    onesq = np.ones((128, 128), np.float16)
    qs = 1.0 / np.sqrt(np.float32(HD))
    g1 = ln1_g[:nl][:, :, None]
    g2 = ln2_g[:nl][:, :, None]

    h16 = lambda x: np.ascontiguousarray(x, dtype=np.float16)
    in_maps = []
    for c in range(NC):
        g, r = c // TP, c % TP
        cols = np.arange(r * QC, (r + 1) * QC)

        wq = g1 * Wattn[:nl][:, :, cols] * qs
        wk = g1 * Wattn[:nl][:, :, D + cols]
        wv_c = g1 * Wattn[:nl][:, :, 2 * D + cols]
        wqk = np.concatenate([wq, wk], axis=2)  # [nl, D, 512]
        csqk = wqk.sum(axis=1, keepdims=True)
        csv = wv_c.sum(axis=1, keepdims=True)

        kiv = ki[:, g, cols][:, :, None]  # [nl, 256, 1]
        vic = vi[:, g, cols]  # [nl, 256]
        viv = np.zeros((nl, 1, NH * 65), np.float32)
        for h in range(NH):
            viv[:, 0, h * 65:h * 65 + 64] = vic[:, h * 64:(h + 1) * 64]
            viv[:, 0, h * 65 + 64] = 1.0

        wproj_c = np.ascontiguousarray(Wproj[:nl][:, cols, :])
        wfc_c = g2 * Wfc[:nl][:, :, r * DFS:(r + 1) * DFS]
        csfc_c = wfc_c.sum(axis=1, keepdims=True)
        wfc2_c = np.ascontiguousarray(Wfc2[:nl][:, r * DFS:(r + 1) * DFS, :])

        ambc = h16(am[g].reshape(4, 128).T)  # [128, 4]

        v0 = r * VSH
        v1 = min(V, v0 + VSH)
        wteT_c = np.zeros((D, VS), np.float16)
        wteT_c[:, : v1 - v0] = h16((wte[v0:v1] * lnf_g[None, :]).T)

        m = {
            "h0T": h16(h0T[:, g * S:(g + 1) * S]),
            "wqk": h16(wqk), "csqk": h16(csqk),
            "wv": h16(wv_c), "csv": h16(csv),
            "kiv": h16(kiv), "viv": h16(viv),
            "wproj": h16(wproj_c),
            "wfc": h16(wfc_c), "csfc": h16(csfc_c),
            "wfc2": h16(wfc2_c),
            "tri": tri, "ambc": ambc, "onesq": onesq,
            "wteT": wteT_c,
        }
        in_maps.append(m)
    return in_maps, nl


_LAST_RESULTS = {}


def kernel(**inputs):
    in_maps, nl = _prep(inputs)
    nc = _build(nl)
    trace = bool(int(os.environ.get("BASS_KERNEL_TRACE", "0")))
    res = bass_utils.run_bass_kernel_spmd(
        nc, in_maps, core_ids=list(range(NC)), trace=trace)
    _LAST_RESULTS["res"] = res
    logits = np.empty((B * S, V), np.float32)
    for c in range(NC):
        g, r = c // TP, c % TP
        v0 = r * VSH
        v1 = min(V, v0 + VSH)
        logits[g * S:(g + 1) * S, v0:v1] = \
            res.results[c]["logits"][:, : v1 - v0].astype(np.float32)
    return logits.reshape(B, S, V)


# revision 23
# speedup vs baseline: 1.1941x; 1.1941x over previous
"""Trainium2 Bass kernel for nn_DecoderModel (12-layer decoder w/ image token).

Sharding: DP2 x TP4.  Cores 0-3 own batch 0, cores 4-7 own batch 1 (512
tokens each).  Megatron TP within each 4-core group:
  - qkv column-sharded (4 heads/core), proj row-sharded + group AllReduce
  - fc column-sharded (1024 dff/core), fc2 row-sharded + group AllReduce
  - lm head: vocab/4 per core for the group's 512 tokens (host assembles)

2-chunk causal pipeline: the 512 tokens are split into two 256-token
chunks.  Chunk 0 attends only to keys 0-255 (+image), so its
attn->proj->AllReduce->ln2->fc->fc2->AllReduce chain is independent of
chunk 1 within a layer; interleaving the two chunks keeps the collective
engine saturated (4 x 0.5MB AllReduce per layer, ~18.8us each = the
pipeline period) while all matmul work hides underneath, and keeps the
PE at max p-state.

Residual kept feature-major (h^T: [D, tok]).  LayerNorm folded into the
matmuls: y = r .* (x @ W - mu * colsum(W)) with gamma folded into W
host-side; the -mu*colsum term is a K=1 matmul into the same PSUM.

Attention: kv order is [tokens 0..511, image] (order inside softmax is
irrelevant).  V is built token-major (tokens on partitions) by swapping
stationary/moving in the matmul.  Scores are kt-major; causal structure
= per-key-tile column slicing plus one shared [128,128] triangle mask on
the diagonal block.  Denominators come from an appended attention-mask
column in V; the 4 per-head denominator broadcasts are fused into 2
block-select matmuls per chunk.
"""

import os
import numpy as np

from concourse import bacc, tile, mybir
from concourse import bass_utils

dt = mybir.dt
AF = mybir.ActivationFunctionType
ALU = mybir.AluOpType

# Model dims (hardcoded per contract)
B, S, D, H, L, V = 2, 512, 1024, 16, 12, 50257
HD = D // H          # 64
DFF = 4 * D          # 4096
NC = 8               # cores
TP = 4               # tensor-parallel group size
TOK = S              # tokens per core (= its batch's 512)
NH = H // TP         # 4 local heads
QC = NH * HD         # 256 q/k/v cols per core
DFS = DFF // TP      # 1024 dff cols per core
PRJ = QC             # 256 proj rows per core
VSH = (V + TP - 1) // TP   # 12565 vocab rows per core
VS = 12800           # padded vocab shard (25*512)
NVT = VS // 512      # 25 vocab tiles
EPS = 1e-5
EXPB = -2.0          # exp(s + EXPB): cancels in normalization; f16 headroom
CW = TOK // 2        # chunk width (tokens per pipeline chunk)

F32 = dt.float32
F16 = dt.float16

GROUPS = [[0, 1, 2, 3], [4, 5, 6, 7]]


def _build(nl):
    nc = bacc.Bacc("TRN2", target_bir_lowering=False, debug=False,
                   num_devices=NC)

    dram = lambda n, sh, ty=F16, kind="ExternalInput": nc.dram_tensor(
        n, sh, ty, kind=kind).ap()

    h0T_d = dram("h0T", [D, TOK])
    wqk_d = dram("wqk", [nl, D, 512])
    csqk_d = dram("csqk", [nl, 1, 512])
    wv_d = dram("wv", [nl, D, QC])
    csv_d = dram("csv", [nl, 1, QC])
    kiv_d = dram("kiv", [nl, QC, 1])
    viv_d = dram("viv", [nl, 1, NH * 65])
    wproj_d = dram("wproj", [nl, PRJ, D])
    wfc_d = dram("wfc", [nl, D, DFS])
    csfc_d = dram("csfc", [nl, 1, DFS])
    wfc2_d = dram("wfc2", [nl, DFS, D])
    tri_d = dram("tri", [128, 128])
    ambc_d = dram("ambc", [128, 4])
    onesq_d = dram("onesq", [128, 128])
    idq_d = dram("idq", [128, 128])      # 0.25 * I  (h/4 fold into ARs)
    wteT_d = dram("wteT", [D, VS])
    logits_d = dram("logits", [TOK, VS], kind="ExternalOutput")

    CR = [slice(0, CW), slice(CW, TOK)]

    with tile.TileContext(nc) as tc:
        with (
            nc.allow_low_precision(reason="f16 pipeline"),
            tc.tile_pool(name="const", bufs=1) as cpool,
            tc.tile_pool(name="resid", bufs=1) as hpool,
            tc.tile_pool(name="rows", bufs=2) as rpool,
            tc.tile_pool(name="dram", bufs=1, space="DRAM") as dpool,
        ):
            ones_sb = cpool.tile([128, 128], F16, name="ones_sb")
            nc.sync.dma_start(ones_sb[:], onesq_d[:])
            ones_col = ones_sb[:, 0:1]
            ones_row = ones_sb[0:1, :]
            tri_sb = cpool.tile([128, 128], F16, name="tri_sb")
            nc.sync.dma_start(tri_sb[:], tri_d[:])
            idq_sb = cpool.tile([128, 128], F16, name="idq_sb")
            nc.sync.dma_start(idq_sb[:], idq_d[:])
            ambsb = cpool.tile([128, 4], F16, name="ambsb")
            nc.sync.dma_start(ambsb[:], ambc_d[:])
            c_eps = cpool.tile([1, 1], F32, name="c_eps")
            nc.vector.memset(c_eps[:], EPS)
            c_invD = cpool.tile([1, 1], F32, name="c_invD")
            nc.vector.memset(c_invD[:], 1.0 / D)
            c_ninvD = cpool.tile([1, 1], F32, name="c_ninvD")
            nc.vector.memset(c_ninvD[:], -1.0 / D)
            c_negb = cpool.tile([128, 1], F32, name="c_negb")
            nc.vector.memset(c_negb[:], EXPB)

            # residual stream, 8 feature chunks [128, TOK]
            hT = []
            for kc in range(8):
                t_ = hpool.tile([128, TOK], F16, name=f"hT{kc}")
                nc.sync.dma_start(t_[:], h0T_d[kc * 128:(kc + 1) * 128, :])
                hT.append(t_)

            # token-major V with per-head [*,65] blocks (col 64 = attn mask)
            v5 = []
            for tc_ in range(4):
                v_ = hpool.tile([128, NH * 65], F16, name=f"v5_{tc_}")
                for h in range(NH):
                    nc.sync.dma_start(v_[:, h * 65 + 64:h * 65 + 65],
                                      ambc_d[:, tc_:tc_ + 1])
                v5.append(v_)

            with (
                tc.tile_pool(name="wts", bufs=2) as wpool,
                tc.tile_pool(name="act", bufs=1) as apool,
                tc.tile_pool(name="scratch", bufs=2) as spool,
                tc.tile_pool(name="ps_mm", bufs=3, space="PSUM") as ps_mm,
                tc.tile_pool(name="ps_s", bufs=3, space="PSUM") as ps_s,
                tc.tile_pool(name="ps_row", bufs=1, space="PSUM") as ps_row,
            ):
                # persistent activation tiles (written/read in chunk slices)
                q_sb = [apool.tile([128, TOK], F16, name=f"q{i}")
                        for i in range(2)]
                kT_sb = [apool.tile([128, S + 1], F16, name=f"kT{i}")
                         for i in range(2)]
                oT_sb = [apool.tile([128, TOK], F16, name=f"oT{i}")
                         for i in range(2)]
                g_sb = [apool.tile([128, TOK], F16, name=f"g{cc}")
                        for cc in range(8)]

                def load_z(arout, c):
                    """hT[:, chunk c] <- arout ([1024, CW] in DRAM).
                    The AR payload carries d_r + h/4 per core, so its sum
                    is the new residual directly (no adds needed)."""
                    cr = CR[c]
                    for kc in range(8):
                        nc.sync.dma_start(hT[kc][:, cr],
                                          arout[kc * 128:(kc + 1) * 128, :])

                def ln_stats(pfx, c, want_rT):
                    """LN stats over hT[:, chunk c].  Returns (nm [1,CW]
                    f16, rb_sb [128,CW] f32, rT_eff [128,2] f32|None)."""
                    cr = CR[c]
                    mu_ps = ps_row.tile([1, CW], F32, tag="rowA", bufs=1)
                    for kc in range(8):
                        nc.tensor.matmul(mu_ps[:], ones_col, hT[kc][:, cr],
                                         start=(kc == 0), stop=(kc == 7))
                    ssq_ps = ps_row.tile([1, CW], F32, tag="rowB", bufs=1)
                    for kc in range(8):
                        xsq = spool.tile([128, CW], F16, tag="xsq", bufs=3)
                        nc.scalar.activation(xsq[:], hT[kc][:, cr],
                                             AF.Square)
                        nc.tensor.matmul(ssq_ps[:], ones_col, xsq[:],
                                         start=(kc == 0), stop=(kc == 7))
                    musq = rpool.tile([1, CW], F32, tag="musq", bufs=2)
                    nc.scalar.activation(musq[:], mu_ps[:], AF.Square,
                                         scale=c_invD[:])
                    varr = rpool.tile([1, CW], F32, tag="varr", bufs=2)
                    nc.vector.scalar_tensor_tensor(
                        varr[:], ssq_ps[:], 1.0 / D, musq[:],
                        ALU.mult, ALU.subtract)
                    sd = rpool.tile([1, CW], F32, tag="sd", bufs=2)
                    nc.scalar.activation(sd[:], varr[:], AF.Sqrt,
                                         bias=c_eps[:])
                    rr = rpool.tile([1, CW], F32, tag="rr", bufs=2)
                    nc.vector.reciprocal_approx_fast(rr[:], sd[:])
                    r16 = rpool.tile([1, CW], F16, tag="r16", bufs=2)
                    nc.scalar.copy(r16[:], rr[:])
                    nm = rpool.tile([1, CW], F16, tag="nm", bufs=4)
                    nc.scalar.mul(nm[:], mu_ps[:], c_ninvD[:])
                    rb_ps = ps_mm.tile([128, CW], F32, tag="mm")
                    nc.tensor.matmul(rb_ps[:], ones_row, r16[:],
                                     start=True, stop=True)
                    rb_sb = spool.tile([128, CW], F32, tag=f"rb{pfx}",
                                       bufs=2)
                    nc.scalar.copy(rb_sb[:], rb_ps[:])
                    rT_eff = None
                    if want_rT:
                        rt_ps = ps_row.tile([128, 2], F32, tag="rowB",
                                            bufs=1)
                        for t in range(2):
                            nc.tensor.matmul(
                                rt_ps[:, t:t + 1],
                                r16[0:1, t * 128:(t + 1) * 128],
                                ones_row[0:1, 0:1],
                                start=True, stop=True,
                                skip_group_check=True)
                        rt_sb = rpool.tile([128, 2], F32, tag="rt", bufs=2)
                        nc.scalar.copy(rt_sb[:], rt_ps[:])
                        rT_eff = rpool.tile([128, 2], F32, tag="rte",
                                            bufs=2)
                        nc.vector.tensor_tensor(
                            rT_eff[:], rt_sb[:],
                            ambsb[:, 2 * c:2 * c + 2], ALU.mult)
                    return nm, rb_sb, rT_eff

                def qkv(l, c, nm1, rb1, rT1, wqk_sb, csqk_sb, wv_sb,
                        csv_sb):
                    cr = CR[c]
                    # q then k chains (each 128 cols of wqk)
                    for cc in range(4):
                        csl = slice(cc * 128, (cc + 1) * 128)
                        ps = ps_mm.tile([128, CW], F32, tag="mm")
                        for kc in range(8):
                            nc.tensor.matmul(ps[:], wqk_sb[kc][:, csl],
                                             hT[kc][:, cr],
                                             start=(kc == 0), stop=False)
                        nc.tensor.matmul(ps[:], csqk_sb[:, csl], nm1[:],
                                         start=False, stop=True)
                        if cc < 2:
                            out = q_sb[cc][:, cr]
                        else:
                            out = kT_sb[cc - 2][:, cr]
                        nc.vector.tensor_tensor(out, ps[:], rb1[:],
                                                ALU.mult)
                    # v chains, token-major (stationary = h token chunk)
                    for t in range(2):
                        tc_ = 2 * c + t
                        tsl = slice(tc_ * 128, (tc_ + 1) * 128)
                        ps = ps_mm.tile([128, CW], F32, tag="mm")
                        for kc in range(8):
                            nc.tensor.matmul(ps[:, 0:QC],
                                             hT[kc][:, tsl], wv_sb[kc][:],
                                             start=(kc == 0), stop=False)
                        nc.tensor.matmul(ps[:, 0:QC],
                                         nm1[0:1, t * 128:(t + 1) * 128],
                                         csv_sb[:], start=False, stop=True)
                        nc.vector.tensor_scalar(
                            v5[tc_].rearrange("p (h w) -> p h w",
                                              h=NH)[:, :, 0:64],
                            ps[:, 0:QC].rearrange("p (h w) -> p h w",
                                                  h=NH),
                            rT1[:, t:t + 1], None, ALU.mult)

                def attn(l, c, viv_sb):
                    """Attention for query chunk c (key tiles 0..2c+1 +
                    image), writing normalized oT slices."""
                    cr = CR[c]
                    nkt = 2 * c + 2      # key tiles visible to this chunk
                    den_ps = {}
                    o_raw = {}
                    p_tiles = {}

                    def scores_head(h):
                        qt = q_sb[h // 2]
                        kt = kT_sb[h // 2]
                        hsl = slice((h % 2) * 64, (h % 2) * 64 + 64)
                        pl = []
                        for ktile in range(nkt):
                            co = max(0, (ktile - 2 * c) * 128)
                            sps = ps_s.tile([128, CW], F32, tag="s")
                            nc.tensor.matmul(
                                sps[:, co:CW],
                                kt[hsl, ktile * 128:(ktile + 1) * 128],
                                qt[hsl, c * CW + co:(c + 1) * CW],
                                start=True, stop=True)
                            p = spool.tile([128, CW], F16, tag="p", bufs=8)
                            if ktile >= 2 * c:
                                # diagonal block: exp then triangle mask
                                ed = spool.tile([128, 128], F16, tag="ed",
                                                bufs=2)
                                nc.scalar.activation(
                                    ed[:], sps[:, co:co + 128],
                                    AF.Exp, bias=c_negb[:])
                                nc.vector.tensor_tensor(
                                    p[:, co:co + 128], ed[:],
                                    tri_sb[:], ALU.mult)
                                if co + 128 < CW:
                                    nc.scalar.activation(
                                        p[:, co + 128:CW],
                                        sps[:, co + 128:CW],
                                        AF.Exp, bias=c_negb[:])
                            else:
                                nc.scalar.activation(
                                    p[:], sps[:], AF.Exp, bias=c_negb[:])
                            pl.append((co, p))
                        simg = ps_row.tile([1, CW], F32,
                                           tag=("rowA", "rowB")[h % 2],
                                           bufs=1)
                        nc.tensor.matmul(simg[:], kt[hsl, S:S + 1],
                                         qt[hsl, cr], start=True,
                                         stop=True)
                        pimg = spool.tile([1, CW], F16, tag="pimg",
                                          bufs=2)
                        nc.scalar.activation(pimg[:], simg[:], AF.Exp,
                                             bias=c_negb[0:1, :])
                        p_tiles[h] = (pl, pimg)

                    def o_head(h):
                        pl, pimg = p_tiles[h]
                        ops = ps_mm.tile([128, CW], F32, tag="mm")
                        for ktile in range(nkt):
                            co, p = pl[ktile]
                            nc.tensor.matmul(
                                ops[0:65, co:CW],
                                v5[ktile][:, h * 65:(h + 1) * 65],
                                p[:, co:CW],
                                start=(ktile == 0), stop=False,
                                skip_group_check=True)
                        nc.tensor.matmul(
                            ops[0:65, :],
                            viv_sb[0:1, h * 65:(h + 1) * 65],
                            pimg[:], start=False, stop=True,
                            skip_group_check=True)
                        oraw = spool.tile([65, CW], F16, tag="oraw",
                                          bufs=4)
                        if h % 2 == 0:
                            nc.scalar.copy(oraw[:], ops[0:65, :])
                        else:
                            nc.vector.tensor_copy(oraw[:], ops[0:65, :])
                        o_raw[h] = oraw

                    scores_head(0)
                    scores_head(1)
                    o_head(0)
                    scores_head(2)
                    o_head(1)
                    scores_head(3)
                    o_head(2)
                    o_head(3)

                    # per-head denominator reciprocal broadcast [64, CW]
                    for h in range(NH):
                        den = rpool.tile([1, CW], F32, tag="den", bufs=2)
                        nc.scalar.copy(den[:], o_raw[h][64:65, :])
                        rcp = rpool.tile([1, CW], F32, tag="rcp", bufs=2)
                        nc.vector.reciprocal_approx_fast(rcp[:], den[:])
                        rch = rpool.tile([1, CW], F16, tag="rch", bufs=2)
                        nc.scalar.copy(rch[:], rcp[:])
                        rbps = ps_mm.tile([128, CW], F32, tag="mm")
                        nc.tensor.matmul(rbps[0:64, :],
                                         ones_row[0:1, 0:64], rch[:],
                                         start=True, stop=True)
                        rbc = spool.tile([64, CW], F32, tag="rbc",
                                         bufs=2)
                        nc.scalar.copy(rbc[:], rbps[0:64, :])
                        hsl = slice((h % 2) * 64, (h % 2) * 64 + 64)
                        nc.vector.tensor_tensor(
                            oT_sb[h // 2][hsl, cr], o_raw[h][0:64, :],
                            rbc[:], ALU.mult)

                def proj_ar(l, c, wproj_sb):
                    cr = CR[c]
                    arin = dpool.tile([D, CW], F16, name=f"aina{l}_{c}")
                    arout = dpool.tile([D, CW], F16, name=f"aouta{l}_{c}")
                    for mc in range(8):
                        msl = slice(mc * 128, (mc + 1) * 128)
                        zps = ps_mm.tile([128, CW], F32, tag="mm")
                        nc.tensor.matmul(zps[:], wproj_sb[0][:, msl],
                                         oT_sb[0][:, cr], start=True,
                                         stop=False)
                        nc.tensor.matmul(zps[:], wproj_sb[1][:, msl],
                                         oT_sb[1][:, cr], start=False,
                                         stop=False)
                        nc.tensor.matmul(zps[:], idq_sb[:],
                                         hT[mc][:, cr], start=False,
                                         stop=True)
                        zsb = spool.tile([128, CW], F16, tag="ardrain",
                                         bufs=4)
                        if mc % 2 == 0:
                            nc.scalar.copy(zsb[:], zps[:])
                        else:
                            nc.vector.tensor_copy(zsb[:], zps[:])
                        nc.sync.dma_start(arin[msl, :], zsb[:])
                    nc.gpsimd.collective_compute(
                        "AllReduce", ALU.add, replica_groups=GROUPS,
                        ins=[arin.opt()], outs=[arout.opt()])
                    return arout

                def fc_fc2_ar(l, c, nm2, rb2, wfc_sb, csfc_sb, wfc2_sb):
                    cr = CR[c]
                    for cc in range(8):
                        csl = slice(cc * 128, (cc + 1) * 128)
                        ps = ps_mm.tile([128, CW], F32, tag="mm")
                        for kc in range(8):
                            nc.tensor.matmul(ps[:], wfc_sb[kc][:, csl],
                                             hT[kc][:, cr],
                                             start=(kc == 0), stop=False)
                        nc.tensor.matmul(ps[:], csfc_sb[:, csl], nm2[:],
                                         start=False, stop=True)
                        pre = spool.tile([128, CW], F32, tag="pre",
                                         bufs=2)
                        nc.vector.tensor_tensor(pre[:], ps[:], rb2[:],
                                                ALU.mult)
                        nc.scalar.activation(g_sb[cc][:, cr], pre[:],
                                             AF.Gelu_apprx_tanh)
                    arin = dpool.tile([D, CW], F16, name=f"ainm{l}_{c}")
                    arout = dpool.tile([D, CW], F16, name=f"aoutm{l}_{c}")
                    for mc in range(8):
                        msl = slice(mc * 128, (mc + 1) * 128)
                        zps = ps_mm.tile([128, CW], F32, tag="mm")
                        for kc in range(8):
                            nc.tensor.matmul(zps[:], wfc2_sb[kc][:, msl],
                                             g_sb[kc][:, cr],
                                             start=(kc == 0), stop=False)
                        nc.tensor.matmul(zps[:], idq_sb[:],
                                         hT[mc][:, cr], start=False,
                                         stop=True)
                        zsb = spool.tile([128, CW], F16, tag="ardrain",
                                         bufs=4)
                        if mc % 2 == 0:
                            nc.scalar.copy(zsb[:], zps[:])
                        else:
                            nc.vector.tensor_copy(zsb[:], zps[:])
                        nc.sync.dma_start(arin[msl, :], zsb[:])
                    nc.gpsimd.collective_compute(
                        "AllReduce", ALU.add, replica_groups=GROUPS,
                        ins=[arin.opt()], outs=[arout.opt()])
                    return arout

                arout_m_prev = [None, None]   # per chunk
                for l in range(nl):
                    # ---- weights for this layer
                    wqk_sb = []
                    for kc in range(8):
                        w = wpool.tile([128, 512], F16, tag=f"wqk{kc}",
                                       name=f"wqk{kc}_{l}")
                        nc.sync.dma_start(
                            w[:], wqk_d[l, kc * 128:(kc + 1) * 128, :])
                        wqk_sb.append(w)
                    csqk_sb = wpool.tile([1, 512], F16, tag="csqk",
                                         name=f"csqk_{l}")
                    nc.sync.dma_start(csqk_sb[:], csqk_d[l])
                    wv_sb = []
                    for kc in range(8):
                        w = wpool.tile([128, QC], F16, tag=f"wv{kc}",
                                       name=f"wv{kc}_{l}")
                        nc.sync.dma_start(
                            w[:], wv_d[l, kc * 128:(kc + 1) * 128, :])
                        wv_sb.append(w)
                    csv_sb = wpool.tile([1, QC], F16, tag="csv",
                                        name=f"csv_{l}")
                    nc.sync.dma_start(csv_sb[:], csv_d[l])
                    viv_sb = wpool.tile([1, NH * 65], F16, tag="viv",
                                        name=f"viv_{l}")
                    nc.sync.dma_start(viv_sb[:], viv_d[l])
                    wproj_sb = []
                    for kc in range(2):
                        w = wpool.tile([128, D], F16, tag=f"wproj{kc}",
                                       name=f"wproj{kc}_{l}")
                        nc.sync.dma_start(
                            w[:], wproj_d[l, kc * 128:(kc + 1) * 128, :])
                        wproj_sb.append(w)
                    wfc_sb = []
                    for kc in range(8):
                        w = wpool.tile([128, DFS], F16, tag=f"wfc{kc}",
                                       name=f"wfc{kc}_{l}")
                        nc.sync.dma_start(
                            w[:], wfc_d[l, kc * 128:(kc + 1) * 128, :])
                        wfc_sb.append(w)
                    csfc_sb = wpool.tile([1, DFS], F16, tag="csfc",
                                         name=f"csfc_{l}")
                    nc.sync.dma_start(csfc_sb[:], csfc_d[l])
                    wfc2_sb = []
                    for kc in range(8):
                        w = wpool.tile([128, D], F16, tag=f"wfc2{kc}",
                                       name=f"wfc2{kc}_{l}")
                        nc.sync.dma_start(
                            w[:], wfc2_d[l, kc * 128:(kc + 1) * 128, :])
                        wfc2_sb.append(w)

                    # image k columns for this layer
                    for i in range(2):
                        nc.sync.dma_start(
                            kT_sb[i][:, S:S + 1],
                            kiv_d[l, i * 128:(i + 1) * 128, :])

                    arout_a = [None, None]
                    # ---- A blocks: resid + ln1 + qkv + attn + proj + AR
                    for c in range(2):
                        if arout_m_prev[c] is not None:
                            load_z(arout_m_prev[c], c)
                        nm1, rb1, rT1 = ln_stats("a", c, True)
                        qkv(l, c, nm1, rb1, rT1, wqk_sb, csqk_sb,
                            wv_sb, csv_sb)
                        attn(l, c, viv_sb)
                        arout_a[c] = proj_ar(l, c, wproj_sb)

                    # ---- B blocks: resid + ln2 + fc + fc2 + AR
                    arout_m = [None, None]
                    for c in range(2):
                        load_z(arout_a[c], c)
                        nm2, rb2, _ = ln_stats("m", c, False)
                        arout_m[c] = fc_fc2_ar(l, c, nm2, rb2, wfc_sb,
                                               csfc_sb, wfc2_sb)
                    arout_m_prev = arout_m

                # ---- final LN (per chunk) -> xf
                xf = [[hpool.tile([128, CW], F16, name=f"xf{c}_{kc}")
                       for kc in range(8)] for c in range(2)]
                for c in range(2):
                    cr = CR[c]
                    load_z(arout_m_prev[c], c)
                    nmf, rbf, _ = ln_stats("f", c, False)
                    nmr = rpool.tile([1, CW], F16, tag="nmr", bufs=2)
                    nc.vector.tensor_tensor(nmr[:], nmf[:],
                                            rbf[0:1, :], ALU.mult)
                    mrb_ps = ps_mm.tile([128, CW], F32, tag="mm")
                    nc.tensor.matmul(mrb_ps[:], ones_row, nmr[:],
                                     start=True, stop=True)
                    mrb = spool.tile([128, CW], F32, tag="mrb", bufs=2)
                    nc.scalar.copy(mrb[:], mrb_ps[:])
                    for kc in range(8):
                        nc.vector.tensor_tensor(xf[c][kc][:],
                                                hT[kc][:, cr], rbf[:],
                                                ALU.mult)
                        nc.vector.tensor_tensor(xf[c][kc][:],
                                                xf[c][kc][:], mrb[:],
                                                ALU.add)

            # ================= LM head =================
            with (
                tc.tile_pool(name="lm_w", bufs=3) as lwpool,
                tc.tile_pool(name="lm_sc", bufs=4) as lspool,
                tc.tile_pool(name="ps_lm", bufs=6, space="PSUM") as ps_lm,
            ):
                for vt in range(NVT):
                    vsl = slice(vt * 512, (vt + 1) * 512)
                    wt_sb = []
                    for kc in range(8):
                        w = lwpool.tile([128, 512], F16, tag=f"wte{kc}",
                                        name=f"wte{kc}_{vt}")
                        nc.sync.dma_start(
                            w[:], wteT_d[kc * 128:(kc + 1) * 128, vsl])
                        wt_sb.append(w)
                    for tcc in range(4):
                        csl = slice(tcc * 128, (tcc + 1) * 128)
                        lsl = slice((tcc % 2) * 128, (tcc % 2 + 1) * 128)
                        lg = ps_lm.tile([128, 512], F32, tag="lg")
                        for kc in range(8):
                            nc.tensor.matmul(lg[:],
                                             xf[tcc // 2][kc][:, lsl],
                                             wt_sb[kc][:],
                                             start=(kc == 0),
                                             stop=(kc == 7))
                        lsb = lspool.tile([128, 512], F16, tag="lmdrain",
                                          bufs=4)
                        if tcc % 2 == 0:
                            nc.scalar.copy(lsb[:], lg[:])
                        else:
                            nc.vector.tensor_copy(lsb[:], lg[:])
                        nc.sync.dma_start(logits_d[csl, vsl], lsb[:])

    nc.compile()
    return nc


def _prep(inputs):
    """Host-side preprocessing. Returns (in_maps, nl)."""
    f = lambda x: np.asarray(x, dtype=np.float32)
    ids = np.asarray(inputs["input_ids"]).astype(np.int64)
    am = f(inputs["attention_mask"])
    ihs = f(inputs["image_hidden_states"])
    wte = f(inputs["wte"])
    ft_W1, ft_b1 = f(inputs["ft_W1"]), f(inputs["ft_b1"])
    ft_W2, ft_b2 = f(inputs["ft_W2"]), f(inputs["ft_b2"])
    ln1_g = f(inputs["ln1_g"])
    Wattn = f(inputs["Wattn"])
    Wuk, buk = f(inputs["Wuk"]), f(inputs["buk"])
    Wuv, buv = f(inputs["Wuv"]), f(inputs["buv"])
    Wproj = f(inputs["Wproj"])
    ln2_g = f(inputs["ln2_g"])
    Wfc = f(inputs["Wfc"])
    Wfc2 = f(inputs["Wfc2"])
    lnf_g = f(inputs["lnf_g"])

    nl = int(os.environ.get("BASS_NLAYERS", str(L)))

    # embedding + image transform
    h0 = wte[ids.reshape(-1)] + np.tile(wte[:S], (B, 1))  # [B*S, D]
    h0T = np.ascontiguousarray(h0.T)
    img = np.maximum(ihs @ ft_W1 + ft_b1, 0.0) @ ft_W2 + ft_b2  # [B, D]

    # image k/v for all layers: [nl, B, D]
    ki = np.einsum("bd,ldm->lbm", img, Wuk[:nl]) + buk[:nl][:, None, :]
    vi = np.einsum("bd,ldm->lbm", img, Wuv[:nl]) + buv[:nl][:, None, :]

    tri = np.triu(np.ones((128, 128), np.float16))
    onesq = np.ones((128, 128), np.float16)
    idq = (0.25 * np.eye(128)).astype(np.float16)
    qs = 1.0 / np.sqrt(np.float32(HD))
    g1 = ln1_g[:nl][:, :, None]
    g2 = ln2_g[:nl][:, :, None]

    h16 = lambda x: np.ascontiguousarray(x, dtype=np.float16)
    in_maps = []
    for c in range(NC):
        g, r = c // TP, c % TP
        cols = np.arange(r * QC, (r + 1) * QC)

        wq = g1 * Wattn[:nl][:, :, cols] * qs
        wk = g1 * Wattn[:nl][:, :, D + cols]
        wv_c = g1 * Wattn[:nl][:, :, 2 * D + cols]
        wqk = np.concatenate([wq, wk], axis=2)  # [nl, D, 512]
        csqk = wqk.sum(axis=1, keepdims=True)
        csv = wv_c.sum(axis=1, keepdims=True)

        kiv = ki[:, g, cols][:, :, None]  # [nl, 256, 1]
        vic = vi[:, g, cols]  # [nl, 256]
        viv = np.zeros((nl, 1, NH * 65), np.float32)
        for h in range(NH):
            viv[:, 0, h * 65:h * 65 + 64] = vic[:, h * 64:(h + 1) * 64]
            viv[:, 0, h * 65 + 64] = 1.0

        wproj_c = np.ascontiguousarray(Wproj[:nl][:, cols, :])
        wfc_c = g2 * Wfc[:nl][:, :, r * DFS:(r + 1) * DFS]
        csfc_c = wfc_c.sum(axis=1, keepdims=True)
        wfc2_c = np.ascontiguousarray(Wfc2[:nl][:, r * DFS:(r + 1) * DFS, :])

        ambc = h16(am[g].reshape(4, 128).T)  # [128, 4]

        v0 = r * VSH
        v1 = min(V, v0 + VSH)
        wteT_c = np.zeros((D, VS), np.float16)
        wteT_c[:, : v1 - v0] = h16((wte[v0:v1] * lnf_g[None, :]).T)

        m = {
            "h0T": h16(h0T[:, g * S:(g + 1) * S]),
            "wqk": h16(wqk), "csqk": h16(csqk),
            "wv": h16(wv_c), "csv": h16(csv),
            "kiv": h16(kiv), "viv": h16(viv),
            "wproj": h16(wproj_c),
            "wfc": h16(wfc_c), "csfc": h16(csfc_c),
            "wfc2": h16(wfc2_c),
            "tri": tri, "ambc": ambc, "onesq": onesq, "idq": idq,
            "wteT": wteT_c,
        }
        in_maps.append(m)
    return in_maps, nl


_LAST_RESULTS = {}


def kernel(**inputs):
    in_maps, nl = _prep(inputs)
    nc = _build(nl)
    trace = bool(int(os.environ.get("BASS_KERNEL_TRACE", "0")))
    res = bass_utils.run_bass_kernel_spmd(
        nc, in_maps, core_ids=list(range(NC)), trace=trace)
    _LAST_RESULTS["res"] = res
    logits = np.empty((B * S, V), np.float32)
    for c in range(NC):
        g, r = c // TP, c % TP
        v0 = r * VSH
        v1 = min(V, v0 + VSH)
        logits[g * S:(g + 1) * S, v0:v1] = \
            res.results[c]["logits"][:, : v1 - v0].astype(np.float32)
    return logits.reshape(B, S, V)
